# revision 15
# baseline (speedup 1.0000x reference)
"""CRF log-partition (forward algorithm) on 8 Trainium2 NeuronCores.

Math: the log-space scan  fv' = logsumexp_prev(fv + trans) + em_t  is run in
LINEAR space:  s' = (E @ s) * x_t  with E = exp(trans), x_t = exp(em_t - c_bt),
where c_bt = logsumexp_l(em[b,t,:]) is a host-side per-(b,t) prescale that keeps
all magnitudes in fp32 range (validated: state stays within [1e-7, 1e-2]).

Parallelism: batch is sharded 8 ways (64 b / core).  Serial depth is halved by
running the forward recursion for t=0..255 and the backward (beta) recursion
for t=511..256 simultaneously; they meet in the middle and are stitched with a
per-b dot product on the host.  On-chip, fwd and bwd are packed into one
128-partition scan: partitions = [fwd: l=0..63 | bwd: l=0..63], so each step is
ONE stationary-weight matmul (W = blockdiag(E^T, E)) + ONE VectorE multiply:

    S_{k+1} = (W^T-apply @ S_k) * X_k      (PSUM fp32 -> SBUF fp32)

The 64 batch elements per core are split into NCH independent chains (free-dim
columns) so PE/DVE pipeline across chains.  The host pre-packs X into the exact
[partition, slot*64+col] layout so the kernel DMAs contiguous slabs and does
zero on-chip transposes, exps, or renormalizations.
"""
import sys

import numpy as np

for _p in ("/opt/trn_rl_repo",):
    if _p not in sys.path:
        sys.path.insert(0, _p)

L = 64
START = L - 2
STOP = L - 1
B = 512
T = 512
NCORES = 8
BPC = B // NCORES      # 64 batch elements per core
Tm = T // 2            # 256 scan slots (fwd+bwd run simultaneously)
NCH = 2                # independent pipeline chains per core
J = BPC // NCH         # free-dim columns per chain
# Ramped X chunk sizes (slots per DMA): tiny first chunks so the scan's first
# tensor_tensor only waits on a 128KB transfer instead of 1MB.
CHUNKS = (2, 2, 4, 8, 16, 32, 64, 64, 64)
assert sum(CHUNKS) == Tm
CHUNK_OFF = tuple(sum(CHUNKS[:i]) for i in range(len(CHUNKS)))

_cached = {}


def _build_bass():
    import concourse.bacc as bacc
    import concourse.mybir as mybir
    from concourse import tile

    f32 = mybir.dt.float32
    bf16 = mybir.dt.bfloat16
    # Bacc (not bare Bass): its compile() runs move_matmul_waits_to_ldweights +
    # generate_event_semaphores, which split multi-sem waits to satisfy the
    # TRN2 1-wait-per-instruction ISA encoding limit.
    nc = bacc.Bacc()
    xd = nc.declare_dram_parameter("x", [128, Tm * 64], f32, isOutput=False)
    # w ([128,128]) and s0 ([128,BPC]) packed side-by-side, pre-cast to bf16 on
    # the host: one small DMA, no on-chip casts before the first matmul.
    wsd = nc.declare_dram_parameter("ws", [128, 128 + BPC], bf16, isOutput=False)
    outd = nc.declare_dram_parameter("out", [128, BPC], f32, isOutput=True)

    with tile.TileContext(nc) as tc:
        with (
            tc.tile_pool(name="const", bufs=1) as cpool,
            tc.tile_pool(name="xbuf", bufs=1) as xpool,
            tc.tile_pool(name="state", bufs=4) as spool,
            tc.tile_pool(name="psum", bufs=3, space="PSUM") as ppool,
        ):
            ws = cpool.tile([128, 128 + BPC], bf16, name="ws")
            nc.sync.dma_start(ws[:], wsd[:, :])
            # X chunk DMAs issue from the GpSimd queue (cheap descriptor gen)
            # in parallel with the ws DMA on the sync queue.
            xch = []
            for ci, csz in enumerate(CHUNKS):
                xt = xpool.tile([128, csz * 64], f32, name=f"xc{ci}", tag=f"xc{ci}")
                nc.gpsimd.dma_start(xt[:], xd[:, CHUNK_OFF[ci] * 64:(CHUNK_OFF[ci] + csz) * 64])
                xch.append(xt)
            w = ws[:, 0:128]
            s0 = ws[:, 128:128 + BPC]
            for ci in range(len(CHUNKS)):
                # Absorb the chunk's DMA-queue semaphore into the DVE clock so
                # the steady-state muls stay within the 2-wait TT ISA limit.
                xab = cpool.tile([1, 1], f32, name=f"xab{ci}", tag="xab")
                nc.vector.tensor_copy(xab[:], xch[ci][0:1, 0:1])

            fin = spool.tile([128, BPC], f32, name="fin", tag="fin")
            state = [s0[:, g * J:(g + 1) * J] for g in range(NCH)]
            for k in range(Tm):
                ci = max(i for i in range(len(CHUNKS)) if CHUNK_OFF[i] <= k)
                off = k - CHUNK_OFF[ci]
                for g in range(NCH):
                    ps = ppool.tile([128, J], f32, name=f"ps{g}_{k}", tag=f"ps{g}")
                    nc.tensor.matmul(ps[:], lhsT=w[:], rhs=state[g], start=True, stop=True)
                    xsl = xch[ci][:, off * 64 + g * J: off * 64 + (g + 1) * J]
                    if k == Tm - 1:
                        # Last slot: f32 out, one DMA per chain on separate
                        # queues so each issues as soon as its chain finishes.
                        nc.vector.tensor_mul(fin[:, g * J:(g + 1) * J], ps[:], xsl)
                        dq = nc.gpsimd if g == 0 else nc.sync
                        dq.dma_start(outd[:, g * J:(g + 1) * J], fin[:, g * J:(g + 1) * J])
                    else:
                        ns = spool.tile([128, J], bf16, name=f"st{g}_{k}", tag=f"st{g}")
                        nc.vector.tensor_mul(ns[:], ps[:], xsl)
                        state[g] = ns
    if not nc.is_finalized():
        nc.finalize()   # Bacc: runs wait-splitting + register allocation

    # The stationary weight matrix W never changes across the 512 matmuls, but
    # Bacc emits an InstLdweights before every InstMatmult (~230ns each on PE,
    # half of all PE time). Keep only the first load; the PE array retains the
    # weights across matmuls. (The removed LDWs carry no sync waits.)
    for blk in nc.m.functions[0].blocks:
        il = list(blk.instructions)
        keep, seen = [], 0
        for i in il:
            if type(i).__name__ == "InstLdweights":
                si = i.sync_info
                has_sync = si is not None and (len(si.on_wait) > 0 or len(si.on_update) > 0)
                seen += 1
                if seen > 1 and not has_sync:
                    continue
            keep.append(i)
        if len(keep) != len(il):
            blk.instructions = keep
    return nc


def _prepare_host(input, transitions):
    em = np.asarray(input, dtype=np.float32)          # [B,T,L]
    trans = np.asarray(transitions, dtype=np.float32)
    E = np.exp(trans.astype(np.float64))              # exp(-1e4) underflows to 0
    Ef = E.astype(np.float32)

    m = em.max(axis=2, keepdims=True)
    c = np.log(np.exp(em - m).sum(axis=2, keepdims=True)) + m   # [B,T,1] f32
    X = np.exp(em - c)                                          # [B,T,L] f32
    csum = c.astype(np.float64).sum(axis=(1, 2))                # [B]

    W = np.zeros((128, 128), np.float32)
    W[0:64, 0:64] = Ef.T        # fwd block: out_top = E @ S_top
    W[64:128, 64:128] = Ef      # bwd block: out_bot = E^T @ S_bot
    Estop = Ef[STOP, :]         # [64]

    in_maps = []
    for cidx in range(NCORES):
        Xc = X[cidx * BPC:(cidx + 1) * BPC]           # [64, T, L]  (b_local, t, l)
        XH = np.empty((Tm, 128, BPC), np.float32)     # [slot, partition, col=b_local]
        # fwd top half: slot k multiplies by x_{t=k}
        XH[:, 0:64, :] = Xc[:, 0:Tm, :].transpose(1, 2, 0)
        # bwd bottom half: slot k multiplies by x_{t=510-k}; slot 255 = ones
        tidx = 510 - np.arange(Tm - 1)
        XH[0:Tm - 1, 64:128, :] = Xc[:, tidx, :].transpose(1, 2, 0)
        XH[Tm - 1, 64:128, :] = 1.0
        xflat = np.ascontiguousarray(
            XH.transpose(1, 0, 2).reshape(128, Tm * BPC))

        s0 = np.zeros((128, BPC), np.float32)
        s0[START, :] = 1.0                            # fwd init: one-hot START
        s0[64:128, :] = (Xc[:, T - 1, :] * Estop).T   # bwd init: x_{511} * E[STOP,:]
        import ml_dtypes
        ws = np.concatenate([W, s0], axis=1).astype(ml_dtypes.bfloat16)
        in_maps.append({"x": xflat, "ws": ws})
    return in_maps, csum


def _stitch(results, csum):
    Z = np.empty(B, np.float64)
    for cidx in range(NCORES):
        out = results[cidx]["out"].astype(np.float64)   # [128, 64]
        dot = (out[0:64] * out[64:128]).sum(axis=0)     # [64] col = b_local
        Z[cidx * BPC:(cidx + 1) * BPC] = np.log(dot) + csum[cidx * BPC:(cidx + 1) * BPC]
    return Z.astype(np.float32)


def _enable_ldw_opt():
    """Flip walrus --enable-ldw-opt to true so the constant stationary weight
    matrix is loaded into the PE array once instead of per-matmul (the scan
    reuses one W for all 512 matmuls; the per-MM LDWEIGHTS otherwise costs
    ~230ns each)."""
    import os
    if os.environ.get("BASS_LDW_OPT") != "1":
        return   # default off: we de-dup LDWEIGHTS ourselves post-finalize
    from concourse import bass_utils
    if getattr(bass_utils.run_command, "_ldw_patched", False):
        return
    orig = bass_utils.run_command

    def patched(argv, **kwargs):
        argv = [a.replace("--enable-ldw-opt=false", "--enable-ldw-opt=true")
                if isinstance(a, str) else a for a in argv]
        return orig(argv, **kwargs)

    patched._ldw_patched = True
    bass_utils.run_command = patched


def _run(input, transitions, trace=False):
    _enable_ldw_opt()
    from concourse.bass_utils import run_bass_kernel_spmd

    if "nc" not in _cached:
        _cached["nc"] = _build_bass()
    nc = _cached["nc"]
    in_maps, csum = _prepare_host(input, transitions)
    res = run_bass_kernel_spmd(nc, in_maps, core_ids=list(range(NCORES)), trace=trace)
    return _stitch(res.results, csum), res


def kernel(input, transitions):
    out, _ = _run(input, transitions, trace=False)
    return out



# revision 16
# speedup vs baseline: 1.0011x; 1.0011x over previous
"""CRF log-partition (forward algorithm) on 8 Trainium2 NeuronCores.

Math: the log-space scan  fv' = logsumexp_prev(fv + trans) + em_t  is run in
LINEAR space:  s' = (E @ s) * x_t  with E = exp(trans), x_t = exp(em_t - c_bt),
where c_bt = logsumexp_l(em[b,t,:]) is a host-side per-(b,t) prescale that keeps
all magnitudes in fp32 range (validated: state stays within [1e-7, 1e-2]).

Parallelism: batch is sharded 8 ways (64 b / core).  Serial depth is halved by
running the forward recursion for t=0..255 and the backward (beta) recursion
for t=511..256 simultaneously; they meet in the middle and are stitched with a
per-b dot product on the host.  On-chip, fwd and bwd are packed into one
128-partition scan: partitions = [fwd: l=0..63 | bwd: l=0..63], so each step is
ONE stationary-weight matmul (W = blockdiag(E^T, E)) + ONE VectorE multiply:

    S_{k+1} = (W^T-apply @ S_k) * X_k      (PSUM fp32 -> SBUF fp32)

The 64 batch elements per core are split into NCH independent chains (free-dim
columns) so PE/DVE pipeline across chains.  The host pre-packs X into the exact
[partition, slot*64+col] layout so the kernel DMAs contiguous slabs and does
zero on-chip transposes, exps, or renormalizations.
"""
import sys

import numpy as np

for _p in ("/opt/trn_rl_repo",):
    if _p not in sys.path:
        sys.path.insert(0, _p)

L = 64
START = L - 2
STOP = L - 1
B = 512
T = 512
NCORES = 8
BPC = B // NCORES      # 64 batch elements per core
Tm = T // 2            # 256 scan slots (fwd+bwd run simultaneously)
NCH = 2                # independent pipeline chains per core
J = BPC // NCH         # free-dim columns per chain
# Ramped X chunk sizes (slots per DMA): tiny first chunks so the scan's first
# tensor_tensor only waits on a 128KB transfer instead of 1MB.
CHUNKS = (4, 4, 8, 16, 32, 64, 64, 64)
assert sum(CHUNKS) == Tm
CHUNK_OFF = tuple(sum(CHUNKS[:i]) for i in range(len(CHUNKS)))

_cached = {}


def _build_bass():
    import concourse.bacc as bacc
    import concourse.mybir as mybir
    from concourse import tile

    f32 = mybir.dt.float32
    bf16 = mybir.dt.bfloat16
    # Bacc (not bare Bass): its compile() runs move_matmul_waits_to_ldweights +
    # generate_event_semaphores, which split multi-sem waits to satisfy the
    # TRN2 1-wait-per-instruction ISA encoding limit.
    nc = bacc.Bacc()
    xd = nc.declare_dram_parameter("x", [128, Tm * 64], f32, isOutput=False)
    # w ([128,128]) and s0 ([128,BPC]) packed side-by-side, pre-cast to bf16 on
    # the host: one small DMA, no on-chip casts before the first matmul.
    wsd = nc.declare_dram_parameter("ws", [128, 128 + BPC], bf16, isOutput=False)
    outd = nc.declare_dram_parameter("out", [128, BPC], f32, isOutput=True)

    with tile.TileContext(nc) as tc:
        with (
            tc.tile_pool(name="const", bufs=1) as cpool,
            tc.tile_pool(name="xbuf", bufs=1) as xpool,
            tc.tile_pool(name="state", bufs=4) as spool,
            tc.tile_pool(name="psum", bufs=3, space="PSUM") as ppool,
        ):
            ws = cpool.tile([128, 128 + BPC], bf16, name="ws")
            nc.sync.dma_start(ws[:], wsd[:, :])
            # X chunk DMAs issue from the GpSimd queue (cheap descriptor gen)
            # in parallel with the ws DMA on the sync queue.
            xch = []
            for ci, csz in enumerate(CHUNKS):
                xt = xpool.tile([128, csz * 64], f32, name=f"xc{ci}", tag=f"xc{ci}")
                nc.gpsimd.dma_start(xt[:], xd[:, CHUNK_OFF[ci] * 64:(CHUNK_OFF[ci] + csz) * 64])
                xch.append(xt)
            w = ws[:, 0:128]
            s0 = ws[:, 128:128 + BPC]
            for ci in range(len(CHUNKS)):
                # Absorb the chunk's DMA-queue semaphore into the DVE clock so
                # the steady-state muls stay within the 2-wait TT ISA limit.
                xab = cpool.tile([1, 1], f32, name=f"xab{ci}", tag="xab")
                nc.vector.tensor_copy(xab[:], xch[ci][0:1, 0:1])

            fin = spool.tile([128, BPC], f32, name="fin", tag="fin")
            state = [s0[:, g * J:(g + 1) * J] for g in range(NCH)]
            for k in range(Tm):
                ci = max(i for i in range(len(CHUNKS)) if CHUNK_OFF[i] <= k)
                off = k - CHUNK_OFF[ci]
                for g in range(NCH):
                    ps = ppool.tile([128, J], f32, name=f"ps{g}_{k}", tag=f"ps{g}")
                    nc.tensor.matmul(ps[:], lhsT=w[:], rhs=state[g], start=True, stop=True)
                    xsl = xch[ci][:, off * 64 + g * J: off * 64 + (g + 1) * J]
                    if k == Tm - 1:
                        # Last slot: f32 out, one DMA per chain on separate
                        # queues so each issues as soon as its chain finishes.
                        nc.vector.tensor_mul(fin[:, g * J:(g + 1) * J], ps[:], xsl)
                        dq = nc.gpsimd if g == 0 else nc.sync
                        dq.dma_start(outd[:, g * J:(g + 1) * J], fin[:, g * J:(g + 1) * J])
                    else:
                        ns = spool.tile([128, J], bf16, name=f"st{g}_{k}", tag=f"st{g}")
                        nc.vector.tensor_mul(ns[:], ps[:], xsl)
                        state[g] = ns
    if not nc.is_finalized():
        nc.finalize()   # Bacc: runs wait-splitting + register allocation

    # The stationary weight matrix W never changes across the 512 matmuls, but
    # Bacc emits an InstLdweights before every InstMatmult (~230ns each on PE,
    # half of all PE time). Keep only the first load; the PE array retains the
    # weights across matmuls. (The removed LDWs carry no sync waits.)
    for blk in nc.m.functions[0].blocks:
        il = list(blk.instructions)
        keep, seen = [], 0
        for i in il:
            if type(i).__name__ == "InstLdweights":
                si = i.sync_info
                has_sync = si is not None and (len(si.on_wait) > 0 or len(si.on_update) > 0)
                seen += 1
                if seen > 1 and not has_sync:
                    continue
            keep.append(i)
        if len(keep) != len(il):
            blk.instructions = keep
    return nc


def _prepare_host(input, transitions):
    em = np.asarray(input, dtype=np.float32)          # [B,T,L]
    trans = np.asarray(transitions, dtype=np.float32)
    E = np.exp(trans.astype(np.float64))              # exp(-1e4) underflows to 0
    Ef = E.astype(np.float32)

    m = em.max(axis=2, keepdims=True)
    c = np.log(np.exp(em - m).sum(axis=2, keepdims=True)) + m   # [B,T,1] f32
    X = np.exp(em - c)                                          # [B,T,L] f32
    csum = c.astype(np.float64).sum(axis=(1, 2))                # [B]

    W = np.zeros((128, 128), np.float32)
    W[0:64, 0:64] = Ef.T        # fwd block: out_top = E @ S_top
    W[64:128, 64:128] = Ef      # bwd block: out_bot = E^T @ S_bot
    Estop = Ef[STOP, :]         # [64]

    in_maps = []
    for cidx in range(NCORES):
        Xc = X[cidx * BPC:(cidx + 1) * BPC]           # [64, T, L]  (b_local, t, l)
        XH = np.empty((Tm, 128, BPC), np.float32)     # [slot, partition, col=b_local]
        # fwd top half: slot k multiplies by x_{t=k}
        XH[:, 0:64, :] = Xc[:, 0:Tm, :].transpose(1, 2, 0)
        # bwd bottom half: slot k multiplies by x_{t=510-k}; slot 255 = ones
        tidx = 510 - np.arange(Tm - 1)
        XH[0:Tm - 1, 64:128, :] = Xc[:, tidx, :].transpose(1, 2, 0)
        XH[Tm - 1, 64:128, :] = 1.0
        xflat = np.ascontiguousarray(
            XH.transpose(1, 0, 2).reshape(128, Tm * BPC))

        s0 = np.zeros((128, BPC), np.float32)
        s0[START, :] = 1.0                            # fwd init: one-hot START
        s0[64:128, :] = (Xc[:, T - 1, :] * Estop).T   # bwd init: x_{511} * E[STOP,:]
        import ml_dtypes
        ws = np.concatenate([W, s0], axis=1).astype(ml_dtypes.bfloat16)
        in_maps.append({"x": xflat, "ws": ws})
    return in_maps, csum


def _stitch(results, csum):
    Z = np.empty(B, np.float64)
    for cidx in range(NCORES):
        out = results[cidx]["out"].astype(np.float64)   # [128, 64]
        dot = (out[0:64] * out[64:128]).sum(axis=0)     # [64] col = b_local
        Z[cidx * BPC:(cidx + 1) * BPC] = np.log(dot) + csum[cidx * BPC:(cidx + 1) * BPC]
    return Z.astype(np.float32)


def _enable_ldw_opt():
    """Flip walrus --enable-ldw-opt to true so the constant stationary weight
    matrix is loaded into the PE array once instead of per-matmul (the scan
    reuses one W for all 512 matmuls; the per-MM LDWEIGHTS otherwise costs
    ~230ns each)."""
    import os
    if os.environ.get("BASS_LDW_OPT") != "1":
        return   # default off: we de-dup LDWEIGHTS ourselves post-finalize
    from concourse import bass_utils
    if getattr(bass_utils.run_command, "_ldw_patched", False):
        return
    orig = bass_utils.run_command

    def patched(argv, **kwargs):
        argv = [a.replace("--enable-ldw-opt=false", "--enable-ldw-opt=true")
                if isinstance(a, str) else a for a in argv]
        return orig(argv, **kwargs)

    patched._ldw_patched = True
    bass_utils.run_command = patched


def _run(input, transitions, trace=False):
    _enable_ldw_opt()
    from concourse.bass_utils import run_bass_kernel_spmd

    if "nc" not in _cached:
        _cached["nc"] = _build_bass()
    nc = _cached["nc"]
    in_maps, csum = _prepare_host(input, transitions)
    res = run_bass_kernel_spmd(nc, in_maps, core_ids=list(range(NCORES)), trace=trace)
    return _stitch(res.results, csum), res


def kernel(input, transitions):
    out, _ = _run(input, transitions, trace=False)
    return out

